# revision 72
# baseline (speedup 1.0000x reference)
"""GCN layer (message passing + linear + ReLU) on 8 Trainium2 NeuronCores.

out = relu(((scatter_add(h[src] -> dst) + x) * dis) @ W.T),
h = x * dis,  dis = rsqrt(deg + 1),  deg = in-degree via dst counts.

Strategy (SPMD, one program on 8 cores):
  - Nodes sharded contiguously: core c owns rows [c*6250, (c+1)*6250).
  - Host partitions edges by dst owner and sorts by dst (index-only work);
    degree reaches the device as CSR rowptr slices, so deg = rowptr diff
    and dis = 1/sqrt(deg+1) are computed on device in f32.
  - The gather table is h = x*dis in bf16 (host-prescaled, O(N) prep like
    the reference's h): each edge's h[src] is one 256B dma_gather row, so
    the scatter matrix S is a pure one-hot (slot==iota) built with a
    single DVE is_equal per group -- no per-edge scale multiply.
  - Scatter-add on-chip: edges sorted by dst fall into windows of 128
    owned nodes; per 128-edge chunk S[e, slot] = (slot==dst-base) is
    built on DVE and the PE accumulates gt.T @ S = agg.T [feat, slot]
    into the window's PSUM tile.
  - Windows are processed in GROUPS of WG: within a (group, pass) the
    chunk runs of all WG windows are packed back-to-back so gather calls
    (<=1024 idx each) span window boundaries.  This cuts the number of
    dma_gather instructions from ~131 to ~85: every Q7 core pair walks
    the whole GpSimd instruction stream (non-owning pairs pay an
    idle-skip per instruction), so fewer+fuller calls raise the pairs'
    useful descriptor-generation throughput, which paces the kernel.
  - Edges inside each (pass, window) block are sorted by src so the
    random 256B HBM reads of a call walk ascending addresses (better
    row-buffer/bank locality on the drain side).
  - int16 gather indices can't span 50k rows, so each window's edges are
    processed against table halves h[:32768] / h[32768:] (pass lo/hi).
  - Gather calls round-robin over 4 SWDGE queues (queue q is served by Q7
    cores 2q/2q+1 -- tx/rx descriptor streams in parallel).
  - Pad gather slots use idx 0 (a real row, masked by S=0).  -1 pad +
    ucode trailing-trim was tried and WEDGES THE DEVICE: the NX decode
    reserves ring space from num_idxs_reg (untrimmed) while the Q7
    writes fewer descriptors, desyncing the SWDGE ring tail so SDMA
    executes stale descriptor slots.  Trimming needs per-core count
    registers, which stall the Pool decode pipeline (baseline note).
  - gidx/xst uploads are split so the first gather/finalize only waits
    for a small first segment.
  - Measured (NTFF traces): the GpSimd extended-instruction stream is the
    pacer -- the Q7 cluster retires gather instructions near-serially at
    ~2.15ns/idx + ~0.3us/call (dispatches run 6+ calls ahead; SDMA queues
    are ~50% idle).  Tried and NOT better (likely noise-dominated, device
    shows +-10-20us run variance): single_packet=False, 48KB SWDGE
    scratch rings, tapered trailing groups, group-batched output DMAs,
    warmup gathers (actively harmful: they serialize on the cluster).
  - Finalize per window, fused right after its last matmul: att =
    (psum + xT) in bf16, po = att.T @ W.T via PE (no transpose needed:
    agg is feature-major), out = relu(po * dis_dst) via ACT per-partition
    scale, then DMA out.
Chunk counts per (pass, window) are maxed over cores so the single SPMD
program fits every core; shorter cores pad with slot=255 / idx=0 chunks.
"""
import numpy as np
import ml_dtypes

from concourse import bacc, bass, mybir, tile
from concourse.bass_utils import run_bass_kernel_spmd

F32 = mybir.dt.float32
BF16 = mybir.dt.bfloat16
I32 = mybir.dt.int32
I16 = mybir.dt.int16
AF = mybir.ActivationFunctionType
OP = mybir.AluOpType

N = 50000
E = 600000
D = 128
C = 8                      # cores
NPC = N // C               # 6250 nodes per core
WPC = (NPC + 127) // 128   # 49 windows per core
NPAD = WPC * 128           # 6272 padded shard rows
NT_G = (N + 127) // 128    # 391 global node tiles
NROWS = NT_G * 128         # 50048 padded table rows
SPLIT = 32768              # src table split for int16 gather indices
PASS_BOUNDS = [(0, SPLIT), (SPLIT, N)]
GB = 8                     # chunks per dma_gather call (1024 idxs = the
                           # <=64 descs-per-packet ceiling of single_packet
                           # mode; 16 engines x 64 = 1024.  GB=12 with
                           # single_packet=True WEDGES the device.  GB=12 +
                           # single_packet=False runs fine (225.8us) -- the
                           # fewer-calls saving is offset by multi-packet
                           # drain fragmentation, so 1024/single-packet stays
                           # the best measured configuration).
SG = 16                    # chunks per S-group build
NQ = 4                     # SWDGE queues used round-robin
WG = 8                     # windows per processing group (gather-call packing
                           # unit; quad-packed PSUM accumulators)
XST_PARTS = 7              # xst upload split (7 windows each)


def _groups():
    return [list(range(g0, min(g0 + WG, WPC))) for g0 in range(0, WPC, WG)]


def _chunk_layout(K):
    """Chunk layout in (group, pass, window) order.

    Returns cbase[p, w] = global chunk index of window w's pass-p block,
    runs[(g, p)] = (start_chunk, n_chunks) of the packed per-group run,
    and TC = total chunks."""
    K = np.asarray(K)
    cbase = np.zeros((2, WPC), np.int64)
    runs = {}
    cb = 0
    for g, ws in enumerate(_groups()):
        for p in range(2):
            r0 = cb
            for w in ws:
                cbase[p, w] = cb
                cb = int(cb + K[p, w])
            runs[(g, p)] = (int(r0), int(cb - r0))
    return cbase, runs, int(cb)


# ---------------------------------------------------------------- host prep
def _assign_windows(nd0, nd1):
    """Assign a core's NPC nodes to WPC windows (<=128 each).

    Windows take nodes in descending pass-1 degree order, so every core's
    window j has a near-identical S1 sum (order statistics align across
    cores) and K1 = ceil(max S1/128) stays near its floor.  Pass-0 sums are
    then balanced under 1024 per window by swapping nodes with |d1 diff|<=2
    (S1 moves stay well inside the per-chunk slack), so K0 = 8 everywhere.
    This packs total chunks ~10% tighter than per-window maxing of a fixed
    node layout -- directly fewer gather indices for the Q7 cluster."""
    order = np.argsort(-nd1, kind="stable")
    win = np.empty(NPC, np.int64)
    s0 = np.zeros(WPC)
    nd1_sorted = nd1[order]
    i = 0
    while i < NPC:
        j = i
        v = nd1_sorted[i]
        while j < NPC and nd1_sorted[j] == v:
            j += 1
        tie_nodes = order[i:j]
        tie_wins = np.arange(i, j) // 128
        wlist, wcount = np.unique(tie_wins, return_counts=True)
        cap = dict(zip(wlist.tolist(), wcount.tolist()))
        td0 = nd0[tie_nodes]
        for k in np.argsort(-td0):
            best = min((w for w in cap if cap[w] > 0), key=lambda w: s0[w])
            win[tie_nodes[k]] = best
            s0[best] += td0[k]
            cap[best] -= 1
        i = j
    S0 = np.zeros(WPC, np.int64)
    S1 = np.zeros(WPC, np.int64)
    np.add.at(S0, win, nd0)
    np.add.at(S1, win, nd1)
    # S1 caps = each window's own initial 128-ceiling: the d1-sorted
    # grouping aligns window S1 across cores (spread ~±9, many windows
    # exactly AT a boundary), K1 pays the cross-core max, so no swap may
    # push a window past its ceiling.  Feasible partners for the S0 swaps
    # then live in d1-NEIGHBOR windows (same or ±1/±2 d1 value), so the
    # search is by d1-value bucket, not by globally lightest window.
    caps1 = np.maximum((np.ceil(S1 / 128) * 128).astype(np.int64), S1)
    import bisect

    bywin = [[] for _ in range(WPC)]     # nodes per window, d0-ascending
    byvw = {}                            # (d1, w) -> nodes, d0-ascending
    wins_of_val = {}
    for n in range(NPC):
        w = int(win[n])
        bywin[w].append(n)
        byvw.setdefault((int(nd1[n]), w), []).append(n)
        wins_of_val.setdefault(int(nd1[n]), set()).add(w)
    kd0 = lambda n: int(nd0[n])
    for w in range(WPC):
        bywin[w].sort(key=kd0)
    for lst in byvw.values():
        lst.sort(key=kd0)

    def pluck(lst, n):
        lst.remove(n)

    for _ in range(6000):
        wbad = int(np.argmax(S0))
        if S0[wbad] <= 1024:
            break
        best = None
        for npl in bywin[wbad][::-1][:8]:
            v, d0p = int(nd1[npl]), int(nd0[npl])
            for dv in (0, -1, 1, -2, 2):
                vv = v + dv
                for wl in wins_of_val.get(vv, ()):
                    if wl == wbad:
                        continue
                    lst = byvw.get((vv, wl))
                    if not lst:
                        continue
                    nm = lst[0]
                    diff0 = d0p - int(nd0[nm])
                    if diff0 <= 0:
                        continue
                    if S0[wl] + diff0 >= S0[wbad]:
                        continue
                    if S1[wl] - dv > caps1[wl]:
                        continue
                    if S1[wbad] + dv > caps1[wbad]:
                        continue
                    key = S0[wl] + diff0
                    if best is None or key < best[0]:
                        best = (key, npl, nm, wl, v, vv, diff0, dv)
        if best is None:
            break
        _, npl, nm, wl, v, vv, diff0, dv = best
        win[npl], win[nm] = wl, wbad
        pluck(bywin[wbad], npl)
        pluck(bywin[wl], nm)
        bisect.insort(bywin[wl], npl, key=kd0)
        bisect.insort(bywin[wbad], nm, key=kd0)
        pluck(byvw[(v, wbad)], npl)
        pluck(byvw[(vv, wl)], nm)
        bisect.insort(byvw.setdefault((v, wl), []), npl, key=kd0)
        bisect.insort(byvw.setdefault((vv, wbad), []), nm, key=kd0)
        wins_of_val.setdefault(v, set()).add(wl)
        wins_of_val.setdefault(vv, set()).add(wbad)
        S0[wbad] -= diff0
        S0[wl] += diff0
        S1[wbad] += vv - v
        S1[wl] += v - vv
    return win, S0, S1


def host_prep(edge_index):
    src = np.asarray(edge_index[0], dtype=np.int64)
    dst = np.asarray(edge_index[1], dtype=np.int64)
    order = np.argsort(dst, kind="stable")
    ss_all = src[order]
    dd_all = dst[order]
    counts = np.bincount(dst, minlength=N)
    rowptr = np.zeros(N + 1, np.int64)
    rowptr[1:] = np.cumsum(counts)
    dis = 1.0 / np.sqrt(counts.astype(np.float64) + 1.0)  # rsqrt(deg+1)
    d0g = np.bincount(dst[src < SPLIT], minlength=N)
    d1g = np.bincount(dst[src >= SPLIT], minlength=N)

    per_core = []
    need = np.zeros((C, 2, WPC), np.int64)
    assigns = []
    for c in range(C):
        e0, e1 = rowptr[c * NPC], rowptr[(c + 1) * NPC]
        per_core.append((ss_all[e0:e1], dd_all[e0:e1]))
        nodes = np.arange(c * NPC, (c + 1) * NPC)
        win, S0, S1 = _assign_windows(d0g[nodes], d1g[nodes])
        # slot of each local node within its window (stable order)
        o2 = np.argsort(win, kind="stable")
        cnt = np.bincount(win, minlength=WPC)
        ofs = np.zeros(WPC, np.int64)
        np.cumsum(cnt[:-1], out=ofs[1:])
        lslot = np.empty(NPC, np.int64)
        lslot[o2] = np.arange(NPC) - ofs[win[o2]]
        assigns.append((win, lslot))
        need[c, 0], need[c, 1] = S0, S1
    K = np.ceil(need.max(axis=0) / 128).astype(np.int64)  # [2, WPC]

    cbase, runs, TC = _chunk_layout(K)
    TC8 = ((TC + SG - 1) // SG) * SG

    cores = []
    for c in range(C):
        ss, dd = per_core[c]
        win, lslot = assigns[c]
        g = np.zeros(TC * 128, np.int64)
        s = np.full(TC8 * 128, 255, np.int64)
        dl = dd - c * NPC
        for p, (lo, hi) in enumerate(PASS_BOUNDS):
            m = (ss >= lo) & (ss < hi)
            sg = ss[m]                       # global src id
            we = win[dl[m]]
            se = lslot[dl[m]]
            # per-window blocks, src-ascending inside each block
            o3 = np.lexsort((sg, we))
            sg, we, se = sg[o3], we[o3], se[o3]
            cnt = np.bincount(we, minlength=WPC)
            ofs = np.zeros(WPC, np.int64)
            np.cumsum(cnt[:-1], out=ofs[1:])
            pos = cbase[p, we] * 128 + (np.arange(len(sg)) - ofs[we])
            g[pos] = sg - lo                 # index into table half
            s[pos] = se
        d = {}
        # gather idx layout [128, TC*8]: stream pos j at [j%16, j//16],
        # replicated across the 8 groups of 16 partitions (each SWDGE queue's
        # tx/rx Q7 core pair reads its own partition group).
        d["gidx"] = np.tile(g.reshape(-1, 16).T.astype(np.int16), (8, 1)).copy()
        # slot layout [128, TC8]: stream pos j at [j%128, j//128]; values
        # <= 255 are exact in bf16 so the S-build is a single is_equal.
        d["slots"] = s.reshape(-1, 128).T.astype(ml_dtypes.bfloat16).copy()
        # per-slot rowptr pairs / x columns / output row map via the node map
        rows = win * 128 + lslot             # local node -> padded out row
        nmap = np.full(NPAD, -1, np.int64)   # padded row -> global node
        nmap[rows] = np.arange(c * NPC, (c + 1) * NPC)
        rp0m = np.zeros(NPAD, np.int64)
        rp1m = np.zeros(NPAD, np.int64)
        mres = nmap >= 0
        rp0m[mres] = rowptr[nmap[mres]]
        rp1m[mres] = rowptr[nmap[mres] + 1]
        d["rp0s"] = rp0m.reshape(WPC, 128).T.astype(np.int32).copy()
        d["rp1s"] = rp1m.reshape(WPC, 128).T.astype(np.int32).copy()
        d["rows"] = rows
        d["nmap"] = nmap
        cores.append(d)
    return dict(K=K, TC=TC, TC8=TC8, cores=cores, dis=dis)


# ---------------------------------------------------------------- program
def build_program(K):
    K = np.asarray(K)
    cbase, runs, TC = _chunk_layout(K)
    TC8 = ((TC + SG - 1) // SG) * SG
    groups = _groups()

    # owner window of every global chunk (program-fixed)
    owner = np.zeros(TC, np.int64)
    for w in range(WPC):
        for p in range(2):
            owner[cbase[p, w] : cbase[p, w] + K[p, w]] = w

    # gidx upload parts over chunk ranges; boundaries are call-aligned
    # (group starts, plus one split 8 chunks in so the very first gather
    # only waits for a ~16KB upload).
    ngrp = len(groups)
    splits = [s for s in [1, 2, 4] if s < ngrp] + [ngrp]
    bounds = [0, min(GB, runs[(0, 0)][1])]
    for s in splits:
        c = TC if s >= ngrp else runs[(s, 0)][0]
        if c > bounds[-1]:
            bounds.append(c)
    gparts = list(zip(bounds[:-1], bounds[1:]))

    nc = bacc.Bacc(
        None, target_bir_lowering=False, debug=False, num_swdge_queues=NQ
    )

    x_p = nc.dram_tensor("xb", [NROWS, D], BF16, kind="ExternalInput")
    xst_p = nc.dram_tensor("xst", [D, NPAD], BF16, kind="ExternalInput")
    wt_p = nc.dram_tensor("wt", [D, D], BF16, kind="ExternalInput")
    iota8_p = nc.dram_tensor("iota8", [128, 128], BF16, kind="ExternalInput")
    rp0s_p = nc.dram_tensor("rp0s", [128, WPC], I32, kind="ExternalInput")
    rp1s_p = nc.dram_tensor("rp1s", [128, WPC], I32, kind="ExternalInput")
    gidx_p = nc.dram_tensor("gidx", [128, TC * 8], I16, kind="ExternalInput")
    slots_p = nc.dram_tensor("slots", [128, TC8], BF16, kind="ExternalInput")
    out_p = nc.dram_tensor("out", [NPAD, D], BF16, kind="ExternalOutput")

    with tile.TileContext(nc) as tc:
        with (
            tc.tile_pool(name="const", bufs=1) as cpool,
            tc.tile_pool(name="gather", bufs=20) as gpool,
            tc.tile_pool(name="sel", bufs=6) as spool,
            tc.tile_pool(name="fin", bufs=3) as fpool,
            tc.tile_pool(name="psA", bufs=4, space="PSUM") as psA,
            tc.tile_pool(name="psO", bufs=2, space="PSUM") as psO,
        ):
            # --- uploads; gidx parts on the sync queue (first part first so
            # gathers start early), metadata then xst on the scalar queue.
            # slots + iota go first on the scalar queue: they gate every
            # S-group build.
            gidx_sb = {}
            for i, (c0, c1) in enumerate(gparts):
                gt_ = cpool.tile([128, (c1 - c0) * 8], I16, tag=f"gidx{i}")
                gidx_sb[i] = (gt_, c0)

            def part_of_chunk(cc):
                for i, (c0, c1) in enumerate(gparts):
                    if c0 <= cc < c1:
                        return i
                raise AssertionError(cc)

            nc.sync.dma_start(
                gidx_sb[0][0][:], gidx_p[:, gparts[0][0] * 8 : gparts[0][1] * 8]
            )
            sf = cpool.tile([128, TC8], BF16, tag="sf")
            nc.scalar.dma_start(sf[:], slots_p[:])
            # one 128-col iota ramp; the S-build reuses it per chunk via a
            # stride-0 AP axis (512KB -> 32KB of early upload traffic)
            iota8_sb = cpool.tile([128, 128], BF16, tag="iota8")
            nc.scalar.dma_start(iota8_sb[:], iota8_p[:])
            wt_sb = cpool.tile([128, 128], BF16, tag="wt")
            nc.scalar.dma_start(wt_sb[:], wt_p[:])
            r0i = cpool.tile([128, WPC], I32, tag="r0i")
            nc.scalar.dma_start(r0i[:], rp0s_p[:])
            r1i = cpool.tile([128, WPC], I32, tag="r1i")
            nc.scalar.dma_start(r1i[:], rp1s_p[:])

            # dis_dst = 1/sqrt(deg+1) from rowptr diffs, [128, WPC] f32
            r0f = cpool.tile([128, WPC], F32, tag="r0f")
            nc.vector.tensor_copy(r0f[:], r0i[:])
            r1f = cpool.tile([128, WPC], F32, tag="r1f")
            nc.vector.tensor_copy(r1f[:], r1i[:])
            dg = cpool.tile([128, WPC], F32, tag="dg")
            nc.vector.tensor_tensor(out=dg[:], in0=r1f[:], in1=r0f[:], op=OP.subtract)
            nc.vector.tensor_scalar_add(out=dg[:], in0=dg[:], scalar1=1.0)
            rc = cpool.tile([128, WPC], F32, tag="rc")
            nc.vector.reciprocal(rc[:], dg[:])
            dis_s = cpool.tile([128, WPC], F32, tag="dis")
            nc.scalar.activation(dis_s[:], rc[:], AF.Sqrt)

            # remaining gidx parts (sync queue), xst parts (scalar queue)
            for i in range(1, len(gparts)):
                t, c0 = gidx_sb[i]
                nc.sync.dma_start(t[:], gidx_p[:, c0 * 8 : gparts[i][1] * 8])
            xst_sb = cpool.tile([128, NPAD], BF16, tag="xst")
            xw = (WPC + XST_PARTS - 1) // XST_PARTS  # windows per xst part
            for i in range(XST_PARTS):
                a, b = i * xw * 128, min((i + 1) * xw * 128, NPAD)
                nc.scalar.dma_start(xst_sb[:, a:b], xst_p[:, a:b])

            tables = [x_p[0:SPLIT, :], x_p[SPLIT:NROWS, :]]
            out_v = out_p[:].rearrange("(u p) d -> p u d", p=128)

            # --- S group builder: chunk-major S[p, k*128 + c] built per
            # SG-chunk group with one DVE is_equal pass (slots vs iota).
            sgroups = {}

            def build_sgroup(gb_):
                Sw = spool.tile([128, 128 * SG], BF16, tag="S")
                sw = Sw[:]
                o = gb_ * SG
                dims = [sw.ap[0], [128, SG], [1, 128]]  # (k, c) iteration
                outap = bass.AP(sw.tensor, sw.offset, dims)
                in0 = bass.AP(sf.tensor, sf.offset + o, [sf.ap[0], [1, SG], [0, 128]])
                ii = iota8_sb[:]
                in1 = bass.AP(ii.tensor, ii.offset, [ii.ap[0], [0, SG], [1, 128]])
                nc.vector.tensor_tensor(out=outap, in0=in0, in1=in1, op=OP.is_equal)
                sgroups[gb_] = Sw
                return Sw

            nmm = (K[0] + K[1]).astype(np.int64)   # matmuls per window
            mm_done = np.zeros(WPC, np.int64)
            # PSUM tiles are bank-aligned (2KB/partition), so pack FOUR
            # windows' [128,128]f32 accumulators into one [128,512] bank
            # tile; a group of WG=8 windows uses 2 quads.
            quad_meta = {}
            quad_first = {}   # quad -> first chunk in global order; only that
                              # matmul gets start=True.  start clears the
                              # has_written bits of the WHOLE bank, so sibling
                              # windows must rely on per-element first-write
                              # overwrite semantics instead of their own start.
            for gi_, ws_ in enumerate(_groups()):
                for i_, w_ in enumerate(ws_):
                    qk = (gi_, i_ // 4)
                    quad_meta[w_] = (qk, i_ % 4)
                    if K[0, w_] + K[1, w_] > 0:
                        fc = int(cbase[0, w_] if K[0, w_] else cbase[1, w_])
                        quad_first[qk] = min(quad_first.get(qk, fc), fc)
            quads = {}

            def psum_slice(w):
                qk, slot = quad_meta[w]
                ent = quads.get(qk)
                if ent is None:
                    ent = quads[qk] = psA.tile(
                        [128, 512], F32, tag="pacc", name="pacc"
                    )
                return ent[:, slot * 128 : (slot + 1) * 128]

            def finalize(w):
                wsl = slice(w * 128, (w + 1) * 128)
                att = fpool.tile([128, 128], BF16, tag="att")
                if int(nmm[w]):
                    nc.vector.tensor_tensor(
                        out=att[:], in0=psum_slice(w), in1=xst_sb[:, wsl],
                        op=OP.add,
                    )
                else:
                    nc.vector.tensor_copy(att[:], xst_sb[:, wsl])
                po = psO.tile([128, 128], F32, tag="po")
                nc.tensor.matmul(
                    po[:], lhsT=att[:], rhs=wt_sb[:], start=True, stop=True
                )
                ot = fpool.tile([128, 128], BF16, tag="ot")
                nc.scalar.activation(
                    ot[:], po[:], AF.Relu, scale=dis_s[:, w : w + 1]
                )
                nc.sync.dma_start(out_v[:, w, :], ot[:])

            # count-register cache: one MOVE per distinct call size instead
            # of one per call (the per-call MOVEs sit on the Pool decode
            # path and bloat the instruction stream).
            nregs = {}

            def creg(v):
                if v not in nregs:
                    nregs[v] = nc.gpsimd.to_reg(v)
                return nregs[v]

            # NOTE: strict round-robin over the 4 queues is load-bearing:
            # each queue's SDMA ring drains its calls near-serially, so
            # skewing calls toward some queues (tried q0 50%/q1 25%) makes
            # that ring's drain the bottleneck (+157us measured).
            qrr = 0
            for gi, ws in enumerate(groups):
                for p in range(2):
                    r0, rn = runs[(gi, p)]
                    done = 0
                    while done < rn:
                        nch = min(GB, rn - done)
                        cc = r0 + done
                        ptile, pbase = gidx_sb[part_of_chunk(cc)]
                        lofs = (cc - pbase) * 8
                        gt = gpool.tile([128, GB * 128], BF16, tag="gt")
                        gv = gt[:, : nch * 128].rearrange(
                            "p (b e) -> p b e", e=128
                        )
                        nc.gpsimd.dma_gather(
                            gv,
                            tables[p],
                            ptile[:, lofs : lofs + nch * 8],
                            nch * 128,
                            creg(nch * 128),
                            D,
                            queue_num=qrr % NQ,
                        )
                        qrr += 1
                        for k in range(nch):
                            gidx_c = cc + k
                            w = int(owner[gidx_c])
                            gb_, kk = divmod(gidx_c, SG)
                            Sw = sgroups.get(gb_)
                            if Sw is None:
                                Sw = build_sgroup(gb_)
                            nc.tensor.matmul(
                                psum_slice(w),
                                lhsT=gt[:, k * 128 : (k + 1) * 128],
                                rhs=Sw[:, kk * 128 : (kk + 1) * 128],
                                start=bool(gidx_c == quad_first[quad_meta[w][0]]),
                                stop=bool(mm_done[w] == nmm[w] - 1),
                                skip_group_check=True,
                            )
                            mm_done[w] += 1
                            if mm_done[w] == nmm[w]:
                                finalize(w)
                        done += nch
                # windows with zero chunks (nmm==0) still need an output
                for w in ws:
                    if nmm[w] == 0 and mm_done[w] == 0:
                        mm_done[w] = -1
                        finalize(w)

    nc.compile()
    return nc


# ---------------------------------------------------------------- runner
_CACHE = {}


def _get_program(K):
    key = K.tobytes()
    if key not in _CACHE:
        _CACHE[key] = build_program(K)
    return _CACHE[key]


def make_in_maps(x, W, prep):
    x = np.asarray(x, np.float32)
    # gather table = h = x * dis (host-prescaled, O(N) prep)
    h = x * prep["dis"][:, None].astype(np.float32)
    xb = np.zeros((NROWS, D), ml_dtypes.bfloat16)
    xb[:N] = h.astype(ml_dtypes.bfloat16)
    Wt = np.ascontiguousarray(np.asarray(W, np.float32).T).astype(
        ml_dtypes.bfloat16
    )
    # iota8[p, c] = c (one ramp; S-build reuses it per chunk via stride-0)
    iota8 = np.tile(
        np.arange(128, dtype=np.float32)[None, :], (128, 1)
    ).astype(ml_dtypes.bfloat16)
    in_maps = []
    for c in range(C):
        cd = prep["cores"][c]
        # x columns laid out by (window, slot) via the node map
        xst = np.zeros((D, NPAD), ml_dtypes.bfloat16)
        nmap, mres = cd["nmap"], cd["nmap"] >= 0
        xst[:, mres] = x[nmap[mres]].T.astype(ml_dtypes.bfloat16)
        in_maps.append(
            {
                "xb": xb,
                "xst": xst,
                "wt": Wt,
                "iota8": iota8,
                "rp0s": cd["rp0s"],
                "rp1s": cd["rp1s"],
                "gidx": cd["gidx"],
                "slots": cd["slots"],
            }
        )
    return in_maps


def run_spmd(x, edge_index, W, trace=False, **spmd_kwargs):
    prep = host_prep(edge_index)
    nc = _get_program(prep["K"])
    in_maps = make_in_maps(x, W, prep)
    res = run_bass_kernel_spmd(nc, in_maps, list(range(C)), trace=trace, **spmd_kwargs)
    parts = []
    for c in range(C):
        ob = np.asarray(res.results[c]["out"], np.float32).reshape(NPAD, D)
        parts.append(ob[prep["cores"][c]["rows"]])
    return np.concatenate(parts, axis=0), res


def kernel(x, edge_index, N=None, W=None, **_):
    out, _res = run_spmd(np.asarray(x), np.asarray(edge_index), np.asarray(W))
    return out
